# revision 42
# baseline (speedup 1.0000x reference)
"""Trainium2 Bass kernel for nn_AxisSimplestSpline (center-anchored ramp basis,
col-tiled PE).  Measured: 322.6us/core, rel err 6.6e-3 (tol 2e-2); was 888us.

Math (per batch b, axis a), with g = (f - mins)/dx in [0,17) anchored at 8.5:
  est_a(g) = V(8.5) + s_8*(g-8.5) + sum_{k=9..16} d_k*relu(g-k)
                                  + sum_{k=1..8} (-d_k)*min(g-k, 0)
  out[c]   = sum_a pinv[a,c] * est_a  (+ bias, added on host)
This exactly reproduces the reference spline (PWL interp of ys_full on the
integer g-grid); telescoping clamp01 into one-sided ramps needs no min+max
pair per knot, and anchoring at the center keeps every feature bounded by
~8 so fp16 rounding is not amplified by cancellation.

PE: input projection is ONE fp16 matmul per 512-chunk (A*inv_dx folded in);
the 17 ramp matmuls write only 48 of 128 output partitions, so they are
packed pairwise into disjoint PE column groups via tile_position — chain A
accumulates into PSUM partitions 0:48 (col groups 0-1), chain B into 64:112
(groups 2-3); order [A_p h0, B_p h0, A_p h1, B_p h1] alternates groups so
paired MMs stream concurrently and each weight set loads once per tile.
One ACT op drains both halves (cost is free-size, not rows) to fp16; the
host sums A + B + bias in fp32 (half magnitudes <=~9, so fp16 is safe).

Features: ACT makes one fp16 G16 = g-8.5 tile; the G16 tile itself is the
linear feature (free); 14 centered features on DVE via dual-op
tensor_scalar at 4x (~335ns each); only L1 and R16 — boundary-sized
slopes — are computed exactly from the fp32 PSUM f on ACT.
(GPSIMD tensor_scalar measured 14.7us per [128,1024] op — unusable.
PSUM cannot be DMA'd directly; fp16-only raw costs 4.4e-3 rel err,
accepted for one fewer matmul.)

Engine busy (measured, final): PE 304us (92%, 1.3us total gaps — at its
structural floor of ~4.75us/tile: 2 input MMs + 18 concurrent pair-slots),
DVE 301us (91%, 14 features x ~335ns x 64 tiles), ACT 279us (85%);
HAM stays warm (2 transitions/run).  Both PE and DVE are saturated at
their floors; further gains need fewer than 136 feature-values/pixel,
which this algorithm family cannot provide without a fast gather
(GPSIMD gather shares one index list per 16-partition core group — its
topology cannot serve per-(axis,pixel) indices at speed).
"""

import sys

sys.path.insert(0, "/opt/trn_rl_repo")

import numpy as np

import concourse.bacc as bacc
import concourse.mybir as mybir
import concourse.tile as tile
from concourse.bass_utils import run_bass_kernel_spmd

F32 = mybir.dt.float32
F16 = mybir.dt.float16
EPS = 1e-4
B, C, H, W = 8, 3, 1024, 1024
HW = H * W
NA, K = 8, 16
J = 16
NJ = HW // J
FREE = 1024
NSUP = NJ // FREE
NCH = FREE // 512
GC = 8.5  # G16 centering

# production order: the G16 tile itself is the linear feature (free),
# then ACT exact feats, then DVE feats
# (GPSIMD tensor_scalar measured 14.7us per [128,1024] op — unusable)
PROD = (
    [("G", 0, "LIN"), ("L", 1, "ACT"), ("R", 16, "ACT"), ("R", 9, "MIX")]
    + [("R", k, "DVE") for k in range(10, 16)]
    + [("L", k, "DVE") for k in range(2, 9)]
)
C1 = 256  # column split for the MIX feature (ACT cols 0:C1 exact, DVE rest)
NF = len(PROD)  # 17
# MM i consumes PROD[i]; chain A = even i (9 MMs, psum parts 0:48),
# chain B = odd i (8 MMs, parts 64:112)
CHAIN = ["A" if i % 2 == 0 else "B" for i in range(NF)]

_NC_CACHE = {}


def _build_nc():
    nc = bacc.Bacc(None, target_bir_lowering=False, debug=False)
    rawh_t = nc.dram_tensor("rawh", [C, HW], F16, kind="ExternalInput")
    # par cols: 0 = -m-GC (G16 bias); 1 = m+1 (ACT-L k=1, scale=-1);
    # 3 = -m-16 (ACT-R k=16)
    par_t = nc.dram_tensor("par", [128, 5], F32, kind="ExternalInput")
    wfh_t = nc.dram_tensor("wfh", [C * J, 128], F16, kind="ExternalInput")
    wks_t = nc.dram_tensor("wks", [128, NF * C * J], F16, kind="ExternalInput")
    # both PSUM halves (A rows 0:48, garbage 48:64, B rows 64:112) in one tensor
    out2_t = nc.dram_tensor("out2", [112, NJ], F16, kind="ExternalOutput")

    Relu = mybir.ActivationFunctionType.Relu
    Ident = mybir.ActivationFunctionType.Identity
    sub = mybir.AluOpType.subtract
    mx = mybir.AluOpType.max
    mn = mybir.AluOpType.min

    with tile.TileContext(nc) as tc:
        with (
            tc.tile_pool(name="const", bufs=1) as cpool,
            tc.tile_pool(name="io", bufs=4) as iopool,
            tc.tile_pool(name="g16", bufs=4) as gpool,
            tc.tile_pool(name="ff", bufs=22) as fpool,
            tc.tile_pool(name="ob", bufs=6) as obpool,
            tc.tile_pool(name="pf", bufs=2, space="PSUM") as pfpool,
            tc.tile_pool(name="po", bufs=2, space="PSUM") as popool,
        ):
            pT = cpool.tile([128, 5], F32)
            nc.sync.dma_start(out=pT[:], in_=par_t[:])
            wfh = cpool.tile([C * J, 128], F16)
            nc.sync.dma_start(out=wfh[:], in_=wfh_t[:])
            wks = cpool.tile([128, NF * C * J], F16)
            nc.sync.dma_start(out=wks[:], in_=wks_t[:])

            rawh_v = rawh_t.ap().rearrange("c (j n) -> (c j) n", j=J)
            out2_v = out2_t.ap()

            fps = [None] * NSUP
            ops = [None] * NSUP

            def drain(n):
                o = ops[n]
                n0 = n * FREE
                # one ACT op copies both halves (cost = free size, not rows)
                ob = obpool.tile([112, FREE], F16, tag="ob")
                nc.scalar.activation(ob[:], o[0:112], Ident, scale=1.0)
                nc.sync.dma_start(out=out2_v[:, n0 : n0 + FREE], in_=ob[:])

            def load_and_project(n):
                n0 = n * FREE
                r = iopool.tile([C * J, FREE], F16, tag="rhs")
                nc.sync.dma_start(out=r[:], in_=rawh_v[:, n0 : n0 + FREE])
                f = pfpool.tile([128, FREE], F32, tag="fps")
                for h in range(NCH):
                    sl = slice(h * 512, (h + 1) * 512)
                    nc.tensor.matmul(f[:, sl], wfh[:], r[:, sl], start=True, stop=True)
                fps[n] = f

            load_and_project(0)

            for n in range(NSUP):
                if n + 1 < NSUP:
                    load_and_project(n + 1)

                f = fps[n]
                g16 = gpool.tile([128, FREE], F16, tag="g16")
                nc.scalar.activation(g16[:], f[:], Ident, bias=pT[:, 0:1], scale=1.0)
                feats = [None] * NF

                def centered(eng, ft, side, k):
                    if side == "R":
                        mk = (17.0 - k) / 2.0
                        eng.tensor_scalar(
                            out=ft[:], in0=g16[:], scalar1=float(k - GC + mk),
                            scalar2=float(-mk), op0=sub, op1=mx,
                        )
                    else:
                        mk = k / 2.0
                        eng.tensor_scalar(
                            out=ft[:], in0=g16[:], scalar1=float(k - GC - mk),
                            scalar2=float(mk), op0=sub, op1=mn,
                        )

                for i, (side, k, eng) in enumerate(PROD):
                    if eng == "LIN":
                        feats[i] = g16
                    elif eng == "MIX":  # R9 split: ACT exact head, DVE tail
                        ft = fpool.tile([128, FREE], F16, tag="F")
                        nc.scalar.activation(
                            ft[:, 0:C1], f[:, 0:C1], Relu,
                            bias=pT[:, 4:5], scale=1.0,
                        )
                        nc.vector.tensor_scalar(
                            out=ft[:, C1:], in0=g16[:, C1:],
                            scalar1=float(k - GC), scalar2=0.0,
                            op0=sub, op1=mx,
                        )
                        feats[i] = ft
                for i, (side, k, eng) in enumerate(PROD):
                    if eng != "ACT":
                        continue
                    ft = fpool.tile([128, FREE], F16, tag="F")
                    if side == "L":  # max(k-g,0) = Relu(-f + (m+k)); col 1
                        nc.scalar.activation(ft[:], f[:], Relu, bias=pT[:, k : k + 1], scale=-1.0)
                    else:  # R16: Relu(f - m - 16); col 3
                        nc.scalar.activation(ft[:], f[:], Relu, bias=pT[:, 3:4], scale=1.0)
                    feats[i] = ft
                for i, (side, k, eng) in enumerate(PROD):
                    if eng != "DVE":
                        continue
                    ft = fpool.tile([128, FREE], F16, tag="F")
                    centered(nc.vector, ft, side, k)
                    feats[i] = ft

                # previous tile's drains: emitted here so ACT/DVE program
                # order is [feats(n), drain(n-1)] and never blocks on knots(n)
                if n >= 1:
                    drain(n - 1)

                o = popool.tile([128, FREE], F32, tag="ops")
                na = CHAIN.count("A")
                nb = CHAIN.count("B")
                # pair-grouped palindrome: [A_p h0, B_p h0, B_p h1, A_p h1] —
                # consecutive same-tile MMs (B h0->h1) reuse the loaded
                # weights with no intervening LDW, cross-tile MMs overlap
                ia = ib = 0
                for p in range(0, NF, 2):
                    pair = [p] + ([p + 1] if p + 1 < NF else [])
                    order = (
                        [(0, pair[0]), (0, pair[1]), (1, pair[1]), (1, pair[0])]
                        if len(pair) == 2
                        else [(0, pair[0]), (1, pair[0])]
                    )
                    for h, i in order:
                        sl = slice(h * 512, (h + 1) * 512)
                        wk = wks[:, i * C * J : (i + 1) * C * J]
                        if CHAIN[i] == "A":
                            nc.tensor.matmul(
                                o[0 : C * J, sl], wk, feats[i][:, sl],
                                start=(ia <= 1), stop=(ia >= 2 * na - 2),
                                tile_position=(0, 0),
                            )
                            ia += 1
                        else:
                            nc.tensor.matmul(
                                o[64 : 64 + C * J, sl], wk, feats[i][:, sl],
                                start=(ib <= 1), stop=(ib >= 2 * nb - 2),
                                tile_position=(0, 64),
                            )
                            ib += 1
                ops[n] = o

            drain(NSUP - 1)
    nc.compile()
    return nc


def _host_params(raw, ys, A):
    in_maps = []
    jr = lambda x: np.repeat(x, J)
    for b in range(B):
        Ab = A[b].astype(np.float32)
        mins = np.minimum(Ab, 0).sum(axis=0)
        maxs = np.maximum(Ab, 0).sum(axis=0)
        pinv = np.linalg.pinv(Ab).astype(np.float32)  # [8, 3]
        span = (maxs + np.float32(EPS) - mins).astype(np.float32)
        dx = (span / np.float32(K + 1)).astype(np.float32)
        inv_dx = (np.float32(1.0) / dx).astype(np.float32)
        Y = np.concatenate(
            [mins[:, None], ys[b].astype(np.float32), maxs[:, None]], axis=1
        )  # [8, 18]
        s = np.diff(Y, axis=1).astype(np.float32)  # [8, 17]

        # anchor between knots at 8.5: linear term s_8 * (g - 8.5) carried by
        # the G16 feature; ramp coefficients are pure second differences
        a = np.zeros((NA, 17), np.float32)
        c = np.zeros((NA, 17), np.float32)
        for k in range(9, 17):
            a[:, k] = s[:, k] - s[:, k - 1]
        for k in range(1, 9):
            c[:, k] = s[:, k - 1] - s[:, k]

        m = (mins * inv_dx).astype(np.float32)
        par = np.zeros((128, 5), np.float32)
        par[:, 0] = jr(-m - np.float32(GC))
        par[:, 1] = jr(m + 1)
        par[:, 2] = jr(m + 2)
        par[:, 3] = jr(-m - 16)
        par[:, 4] = jr(-m - 9)

        wf = (Ab * inv_dx[None, :]).astype(np.float32)  # [3, 8]
        wfm = np.zeros((C * J, 128), np.float32)
        for jj in range(J):
            for cc in range(C):
                for aa in range(NA):
                    wfm[cc * J + jj, aa * J + jj] = wf[cc, aa]
        wfh = wfm.astype(np.float16)

        bias = ((Y[:, 8] + Y[:, 9]) * np.float32(0.5)).astype(np.float32)  # V(8.5)
        wco = np.zeros((NA, NF), np.float32)
        for i, (side, k, eng) in enumerate(PROD):
            if side == "G":
                wco[:, i] = s[:, 8]
            elif eng == "MIX":
                wco[:, i] = a[:, k]
            elif side == "R":
                wco[:, i] = a[:, k]
                if eng == "DVE":
                    bias += a[:, k] * np.float32((17.0 - k) / 2.0)
            else:
                if eng == "ACT":
                    wco[:, i] = -c[:, k]
                else:
                    wco[:, i] = c[:, k]
                    bias -= c[:, k] * np.float32(k / 2.0)

        wks = np.zeros((128, NF * C * J), np.float16)
        for i in range(NF):
            for jj in range(J):
                for cc in range(C):
                    for aa in range(NA):
                        wks[aa * J + jj, i * C * J + cc * J + jj] = (
                            pinv[aa, cc] * wco[aa, i]
                        )

        b0 = (pinv * bias[:, None]).sum(axis=0)  # [3], added on host

        rb = np.ascontiguousarray(raw[b].reshape(C, HW), np.float32)
        rh = rb.astype(np.float16)
        in_maps.append(
            {
                "rawh": rh,
                "par": par,
                "wfh": wfh,
                "wks": wks,
                "_b0": b0,  # host-side only
            }
        )
    return in_maps


def kernel(raw, ys, A):
    raw = np.asarray(raw, np.float32)
    ys = np.asarray(ys, np.float32)
    A = np.asarray(A, np.float32)
    if "nc" not in _NC_CACHE:
        _NC_CACHE["nc"] = _build_nc()
    nc = _NC_CACHE["nc"]
    in_maps = _host_params(raw, ys, A)
    dev_maps = [{k: v for k, v in im.items() if not k.startswith("_")} for im in in_maps]
    res = run_bass_kernel_spmd(nc, dev_maps, core_ids=list(range(B)))
    outs = []
    for b in range(B):
        o2 = res.results[b]["out2"].astype(np.float32)  # [112, NJ] fp16 halves
        o = o2[0:48] + o2[64:112]
        o = o.reshape(C, J, NJ) + in_maps[b]["_b0"][:, None, None].astype(np.float32)
        outs.append(o.reshape(C, H, W))
    return np.stack(outs).astype(np.float32)
